# revision 4
# baseline (speedup 1.0000x reference)
"""GCN encoder (GIN conv -> 2x GCN conv) on 8 Trainium2 NeuronCores.

Strategy (dst-sharded, graph-parallel):
- Nodes are sharded by dst across 8 cores (12500 each). Each core owns the
  segment-sums and all dense math for its nodes; weights are replicated.
- Within a core, nodes are sorted by in-degree and grouped into 98 blocks of
  128; each block is padded to its max degree D_b, giving a dense
  [D_b, 128, 64] "slot" layout where tile s holds the s-th in-edge message of
  each of the 128 nodes. The segment-sum is then a chain of D_b TensorE
  matmuls accumulating into PSUM (lhsT = message tile, rhs = identity), which
  yields the aggregate directly in feature-major layout for the following
  linear layers.
- Per-edge message rows (x[src] for the GIN pass, the dinv-scaled
  concatenated GCN projections m[src] for the fused mu/logvar pass) are
  materialized into the slot layout on the host as part of input sharding;
  the device consumes them as dense streams at full DMA bandwidth.
- GCN normalization dinv[src]*dinv[dst] is factored: the table rows are
  pre-scaled by dinv[src] on device (launch A epilogue), and the dst factor
  is applied after the segment-sum (launch C epilogue), so no per-edge
  normalization gather is needed:
      out_i = dinv_i * (sum_{j->i} m_j + m_i) + b,   m_j = dinv_j * (h W)_j

Two SPMD launches:
  A: slots1 (x[src] rows) -> agg -> h = relu((x+agg) gin_W + gin_b)
     -> m = dinv * (h [mu_W|lv_W])          (per-core slice, feature-major)
  C: slots2 (m[src] rows) -> segment-sum -> epilogue -> [mu|logvar]
Host between launches: assemble the m table from the 8 slices and gather it
into the pass-2 slot layout (same index structure as pass 1).
"""

import numpy as np
import ml_dtypes

BF16 = ml_dtypes.bfloat16

N = 100000
E = 1600000
CIN = 64
HID = 64
COUT = 32
NCORES = 8
NPC = N // NCORES            # 12500 real nodes per core
BLK = 128
NBLK = 98                    # blocks per core
NPCP = NBLK * BLK            # 12544 padded positions per core

_cache = {}


def _build_programs(d_sched):
    import concourse.bass as bass
    import concourse.bacc as bacc
    import concourse.mybir as mybir
    import concourse.tile as tile
    from concourse.masks import make_identity

    t1 = int(np.sum(d_sched))
    tile_off = np.concatenate([[0], np.cumsum(d_sched)]).astype(int)

    def build(which):
        nc = bacc.Bacc("TRN2", target_bir_lowering=False, debug=False,
                       enable_asserts=False, num_devices=NCORES)
        slots = nc.dram_tensor("slots", [t1, BLK, 64], mybir.dt.bfloat16,
                               kind="ExternalInput").ap()
        selfT = nc.dram_tensor("selfT", [64, NPCP], mybir.dt.float32,
                               kind="ExternalInput").ap()
        dinvT = nc.dram_tensor("dinvT", [64, NPCP], mybir.dt.float32,
                               kind="ExternalInput").ap()
        if which == "A":
            ginW = nc.dram_tensor("ginW", [64, 64], mybir.dt.bfloat16,
                                  kind="ExternalInput").ap()
            ginb = nc.dram_tensor("ginb", [64, 1], mybir.dt.float32,
                                  kind="ExternalInput").ap()
            wcat = nc.dram_tensor("wcat", [64, 64], mybir.dt.bfloat16,
                                  kind="ExternalInput").ap()
            outT = nc.dram_tensor("outT", [64, NPCP], mybir.dt.bfloat16,
                                  kind="ExternalOutput").ap()
        else:
            bias = nc.dram_tensor("bias", [64, 1], mybir.dt.float32,
                                  kind="ExternalInput").ap()
            outT = nc.dram_tensor("outT", [64, NPCP], mybir.dt.float32,
                                  kind="ExternalOutput").ap()

        with tile.TileContext(nc) as tc:
            with (tc.tile_pool(name="const", bufs=1) as cpool,
                  tc.tile_pool(name="blkin", bufs=3) as bpool,
                  tc.tile_pool(name="work", bufs=4) as wpool,
                  tc.tile_pool(name="ps", bufs=3, space="PSUM") as ppool,
                  tc.tile_pool(name="ps2", bufs=2, space="PSUM") as p2pool):
                ident = cpool.tile([BLK, BLK], mybir.dt.bfloat16)
                make_identity(nc, ident[:])
                selfT_sb = cpool.tile([64, NPCP], mybir.dt.float32)
                nc.sync.dma_start(out=selfT_sb[:], in_=selfT[:])
                dinvT_sb = cpool.tile([64, NPCP], mybir.dt.float32)
                nc.sync.dma_start(out=dinvT_sb[:], in_=dinvT[:])
                if which == "A":
                    ginW_sb = cpool.tile([64, 64], mybir.dt.bfloat16)
                    nc.sync.dma_start(out=ginW_sb[:], in_=ginW[:])
                    ginb_sb = cpool.tile([64, 1], mybir.dt.float32)
                    nc.sync.dma_start(out=ginb_sb[:], in_=ginb[:])
                    wcat_sb = cpool.tile([64, 64], mybir.dt.bfloat16)
                    nc.sync.dma_start(out=wcat_sb[:], in_=wcat[:])
                    acc = cpool.tile([64, NPCP], mybir.dt.bfloat16)
                else:
                    bias_sb = cpool.tile([64, 1], mybir.dt.float32)
                    nc.sync.dma_start(out=bias_sb[:], in_=bias[:])
                    acc = cpool.tile([64, NPCP], mybir.dt.float32)

                dmax = int(np.max(d_sched))
                for b in range(NBLK):
                    db = int(d_sched[b])
                    bsl = slice(b * BLK, (b + 1) * BLK)
                    # one DMA brings the whole block's slot tiles:
                    # DRAM [db, 128, 64] -> SBUF [128, db, 64]
                    blkt = bpool.tile([BLK, dmax, 64], mybir.dt.bfloat16,
                                      tag="blk")
                    src_ap = slots[tile_off[b]:tile_off[b] + db]
                    nc.sync.dma_start(
                        out=blkt[:, :db, :],
                        in_=src_ap.rearrange("d p f -> p d f"),
                    )
                    ps = ppool.tile([64, BLK], mybir.dt.float32, space="PSUM")
                    for s in range(db):
                        nc.tensor.matmul(
                            out=ps[:],
                            lhsT=blkt[:, s, :],
                            rhs=ident[:],
                            start=(s == 0),
                            stop=(s == db - 1),
                        )
                    if which == "A":
                        # xin = (x + agg) as bf16, feature-major
                        xin = wpool.tile([64, BLK], mybir.dt.bfloat16,
                                         tag="xin")
                        nc.vector.tensor_add(
                            out=xin[:], in0=ps[:], in1=selfT_sb[:, bsl])
                        ps2 = p2pool.tile([64, BLK], mybir.dt.float32,
                                          space="PSUM")
                        nc.tensor.matmul(out=ps2[:], lhsT=ginW_sb[:],
                                         rhs=xin[:], start=True, stop=True)
                        hT = wpool.tile([64, BLK], mybir.dt.bfloat16,
                                        tag="hT")
                        nc.scalar.activation(
                            hT[:], ps2[:],
                            mybir.ActivationFunctionType.Relu,
                            bias=ginb_sb[:], scale=1.0)
                        ps3 = p2pool.tile([64, BLK], mybir.dt.float32,
                                          space="PSUM")
                        nc.tensor.matmul(out=ps3[:], lhsT=wcat_sb[:],
                                         rhs=hT[:], start=True, stop=True)
                        nc.vector.tensor_mul(
                            out=acc[:, bsl], in0=ps3[:],
                            in1=dinvT_sb[:, bsl])
                    else:
                        t1w = wpool.tile([64, BLK], mybir.dt.float32,
                                         tag="t1")
                        nc.vector.tensor_add(
                            out=t1w[:], in0=ps[:], in1=selfT_sb[:, bsl])
                        t2w = wpool.tile([64, BLK], mybir.dt.float32,
                                         tag="t2")
                        nc.vector.tensor_mul(
                            out=t2w[:], in0=t1w[:], in1=dinvT_sb[:, bsl])
                        # mu rows get relu, logvar rows pass through
                        nc.scalar.activation(
                            acc[0:COUT, bsl], t2w[0:COUT, :],
                            mybir.ActivationFunctionType.Relu,
                            bias=bias_sb[0:COUT, :], scale=1.0)
                        nc.vector.tensor_scalar_add(
                            acc[COUT:64, bsl], t2w[COUT:64, :],
                            bias_sb[COUT:64, :])
                nc.sync.dma_start(out=outT[:], in_=acc[:])
        nc.compile()
        from concourse.bass_interp import get_hw_module
        nc.m = get_hw_module(nc.m)
        return nc

    return build("A"), build("C")


def _prep(edge_index):
    """Shard/sort/pad the graph; returns per-core index structures."""
    src = np.asarray(edge_index[0], dtype=np.int64)
    dst = np.asarray(edge_index[1], dtype=np.int64)
    deg_in = np.bincount(dst, minlength=N)
    dinv = (1.0 / np.sqrt(deg_in + 1.0)).astype(np.float32)

    cores = []
    d_sched_per_core = np.zeros((NCORES, NBLK), dtype=np.int64)
    for c in range(NCORES):
        lo, hi = c * NPC, (c + 1) * NPC
        m = (dst >= lo) & (dst < hi)
        s_c = src[m]
        d_c = (dst[m] - lo).astype(np.int64)
        deg_c = np.bincount(d_c, minlength=NPC)
        order = np.argsort(deg_c, kind="stable")      # position -> local node
        pos = np.empty(NPC, dtype=np.int64)
        pos[order] = np.arange(NPC)                   # local node -> position
        posdeg = np.zeros(NPCP, dtype=np.int64)
        posdeg[:NPC] = deg_c[order]
        d_sched_per_core[c] = posdeg.reshape(NBLK, BLK).max(axis=1)
        cores.append((s_c, d_c, deg_c, order, pos, posdeg))

    d_sched = d_sched_per_core.max(axis=0)
    d_sched = np.maximum(d_sched, 1)
    t1 = int(d_sched.sum())
    tile_off = np.concatenate([[0], np.cumsum(d_sched)]).astype(np.int64)

    srcidx = np.full((NCORES, t1, BLK), -1, dtype=np.int64)
    pos_of_global = np.empty(N, dtype=np.int64)
    for c in range(NCORES):
        s_c, d_c, deg_c, order, pos, posdeg = cores[c]
        pos_of_global[c * NPC + order] = c * NPCP + np.arange(NPC)
        key = pos[d_c]
        eord = np.argsort(key, kind="stable")
        spos = key[eord]
        start_of_pos = np.zeros(NPCP, dtype=np.int64)
        np.cumsum(posdeg[:-1], out=start_of_pos[1:])
        r = np.arange(len(spos)) - start_of_pos[spos]
        t = tile_off[spos // BLK] + r
        srcidx[c, t, spos % BLK] = s_c[eord]
    return d_sched, t1, srcidx, pos_of_global, dinv, cores


TRACE = False
last_exec_ns = []


def _run(nc, in_maps):
    from concourse import bass_utils
    res = bass_utils.run_bass_kernel_spmd(nc, in_maps,
                                          core_ids=list(range(NCORES)),
                                          trace=TRACE)
    if TRACE:
        last_exec_ns.append(res.exec_time_ns)
    return res.results


def kernel(x, edge_index, gin_W, gin_b, mu_W, mu_b, lv_W, lv_b):
    x = np.asarray(x, dtype=np.float32)
    gin_W = np.asarray(gin_W, dtype=np.float32)
    gin_b = np.asarray(gin_b, dtype=np.float32)
    wcat = np.concatenate([np.asarray(mu_W, np.float32),
                           np.asarray(lv_W, np.float32)], axis=1)
    bias_cat = np.concatenate([np.asarray(mu_b, np.float32),
                               np.asarray(lv_b, np.float32)])

    d_sched, t1, srcidx, pos_of_global, dinv, cores = _prep(edge_index)

    key = ("prog", t1, tuple(int(v) for v in d_sched))
    if key not in _cache:
        _cache[key] = _build_programs(d_sched)
    nc_A, nc_C = _cache[key]

    # ---- launch A inputs ----
    x_pad = np.zeros((N + 1, 64), dtype=BF16)
    x_pad[:N] = x.astype(BF16)
    gather1 = np.where(srcidx >= 0, srcidx, N)

    in_maps_A = []
    for c in range(NCORES):
        _, _, _, order, _, _ = cores[c]
        xT = np.zeros((64, NPCP), dtype=np.float32)
        xT[:, :NPC] = x[c * NPC + order].T
        dT = np.ones((NPCP,), dtype=np.float32)
        dT[:NPC] = dinv[c * NPC + order]
        dinvT = np.broadcast_to(dT, (64, NPCP)).copy()
        in_maps_A.append({
            "slots": x_pad[gather1[c]],
            "selfT": xT,
            "dinvT": dinvT,
            "ginW": gin_W.astype(BF16),
            "ginb": gin_b.reshape(64, 1),
            "wcat": wcat.astype(BF16),
        })
    res_A = _run(nc_A, in_maps_A)

    # ---- assemble m table, build launch C inputs ----
    m_pos = np.zeros((NCORES * NPCP + 1, 64), dtype=BF16)
    for c in range(NCORES):
        m_pos[c * NPCP:(c + 1) * NPCP] = res_A[c]["outT"].T
    gather2 = np.where(srcidx >= 0, pos_of_global[srcidx],
                       NCORES * NPCP)

    in_maps_C = []
    for c in range(NCORES):
        in_maps_C.append({
            "slots": m_pos[gather2[c]],
            "selfT": m_pos[c * NPCP:(c + 1) * NPCP].T.astype(np.float32),
            "dinvT": in_maps_A[c]["dinvT"],
            "bias": bias_cat.reshape(64, 1),
        })
    res_C = _run(nc_C, in_maps_C)

    # ---- unshard ----
    out = np.empty((N, 64), dtype=np.float32)
    for c in range(NCORES):
        _, _, _, order, _, _ = cores[c]
        out[c * NPC + order] = res_C[c]["outT"][:, :NPC].T
    return out[:, :COUT], out[:, COUT:]


# revision 5
# speedup vs baseline: 1.2491x; 1.2491x over previous
"""GCN encoder (GIN conv -> 2x GCN conv) on 8 Trainium2 NeuronCores.

Strategy (dst-sharded, graph-parallel):
- Nodes are sharded by dst across 8 cores (12500 each). Each core owns the
  segment-sums and all dense math for its nodes; weights are replicated.
- Within a core, nodes are sorted by in-degree and grouped into 98 blocks of
  128; each block is padded to its max degree D_b, giving a dense
  [D_b, 128, 64] "slot" layout where tile s holds the s-th in-edge message of
  each of the 128 nodes. The segment-sum is then a chain of D_b TensorE
  matmuls accumulating into PSUM (lhsT = message tile, rhs = identity), which
  yields the aggregate directly in feature-major layout for the following
  linear layers.
- Per-edge message rows (x[src] for the GIN pass, the dinv-scaled
  concatenated GCN projections m[src] for the fused mu/logvar pass) are
  materialized into the slot layout on the host as part of input sharding;
  the device consumes them as dense streams at full DMA bandwidth.
- GCN normalization dinv[src]*dinv[dst] is factored: the table rows are
  pre-scaled by dinv[src] on device (launch A epilogue), and the dst factor
  is applied after the segment-sum (launch C epilogue), so no per-edge
  normalization gather is needed:
      out_i = dinv_i * (sum_{j->i} m_j + m_i) + b,   m_j = dinv_j * (h W)_j

Two SPMD launches:
  A: slots1 (x[src] rows) -> agg -> h = relu((x+agg) gin_W + gin_b)
     -> m = dinv * (h [mu_W|lv_W])          (per-core slice, feature-major)
  C: slots2 (m[src] rows) -> segment-sum -> epilogue -> [mu|logvar]
Host between launches: assemble the m table from the 8 slices and gather it
into the pass-2 slot layout (same index structure as pass 1).
"""

import numpy as np
import ml_dtypes

BF16 = ml_dtypes.bfloat16

N = 100000
E = 1600000
CIN = 64
HID = 64
COUT = 32
NCORES = 8
NPC = N // NCORES            # 12500 real nodes per core
BLK = 128
NBLK = 98                    # blocks per core
NPCP = NBLK * BLK            # 12544 padded positions per core

_cache = {}


def _build_programs(d_sched):
    import concourse.bass as bass
    import concourse.bacc as bacc
    import concourse.mybir as mybir
    import concourse.tile as tile
    from concourse.masks import make_identity

    t1 = int(np.sum(d_sched))
    tile_off = np.concatenate([[0], np.cumsum(d_sched)]).astype(int)

    def build(which):
        nc = bacc.Bacc("TRN2", target_bir_lowering=False, debug=False,
                       enable_asserts=False, num_devices=NCORES)
        slots = nc.dram_tensor("slots", [BLK, t1, 64], mybir.dt.bfloat16,
                               kind="ExternalInput").ap()
        selfT = nc.dram_tensor("selfT", [64, NPCP], mybir.dt.float32,
                               kind="ExternalInput").ap()
        dinvT = nc.dram_tensor("dinvT", [64, NPCP], mybir.dt.float32,
                               kind="ExternalInput").ap()
        if which == "A":
            ginW = nc.dram_tensor("ginW", [64, 64], mybir.dt.bfloat16,
                                  kind="ExternalInput").ap()
            ginb = nc.dram_tensor("ginb", [64, 1], mybir.dt.float32,
                                  kind="ExternalInput").ap()
            wcat = nc.dram_tensor("wcat", [64, 64], mybir.dt.bfloat16,
                                  kind="ExternalInput").ap()
            outT = nc.dram_tensor("outT", [64, NPCP], mybir.dt.bfloat16,
                                  kind="ExternalOutput").ap()
        else:
            bias = nc.dram_tensor("bias", [64, 1], mybir.dt.float32,
                                  kind="ExternalInput").ap()
            outT = nc.dram_tensor("outT", [64, NPCP], mybir.dt.float32,
                                  kind="ExternalOutput").ap()

        with tile.TileContext(nc) as tc:
            with (tc.tile_pool(name="const", bufs=1) as cpool,
                  tc.tile_pool(name="blkin", bufs=3) as bpool,
                  tc.tile_pool(name="work", bufs=4) as wpool,
                  tc.tile_pool(name="ps", bufs=3, space="PSUM") as ppool,
                  tc.tile_pool(name="ps2", bufs=2, space="PSUM") as p2pool):
                ident = cpool.tile([BLK, BLK], mybir.dt.bfloat16)
                make_identity(nc, ident[:])
                selfT_sb = cpool.tile([64, NPCP], mybir.dt.float32)
                nc.sync.dma_start(out=selfT_sb[:], in_=selfT[:])
                dinvT_sb = cpool.tile([64, NPCP], mybir.dt.float32)
                nc.sync.dma_start(out=dinvT_sb[:], in_=dinvT[:])
                if which == "A":
                    ginW_sb = cpool.tile([64, 64], mybir.dt.bfloat16)
                    nc.sync.dma_start(out=ginW_sb[:], in_=ginW[:])
                    ginb_sb = cpool.tile([64, 1], mybir.dt.float32)
                    nc.sync.dma_start(out=ginb_sb[:], in_=ginb[:])
                    wcat_sb = cpool.tile([64, 64], mybir.dt.bfloat16)
                    nc.sync.dma_start(out=wcat_sb[:], in_=wcat[:])
                    acc = cpool.tile([64, NPCP], mybir.dt.bfloat16)
                else:
                    bias_sb = cpool.tile([64, 1], mybir.dt.float32)
                    nc.sync.dma_start(out=bias_sb[:], in_=bias[:])
                    acc = cpool.tile([64, NPCP], mybir.dt.float32)

                dmax = int(np.max(d_sched))
                for b in range(NBLK):
                    db = int(d_sched[b])
                    bsl = slice(b * BLK, (b + 1) * BLK)
                    # one DMA brings the whole block's slot tiles:
                    # DRAM [db, 128, 64] -> SBUF [128, db, 64]
                    blkt = bpool.tile([BLK, dmax, 64], mybir.dt.bfloat16,
                                      tag="blk")
                    nc.sync.dma_start(
                        out=blkt[:, :db, :],
                        in_=slots[:, tile_off[b]:tile_off[b] + db, :],
                    )
                    ps = ppool.tile([64, BLK], mybir.dt.float32, space="PSUM")
                    for s in range(db):
                        nc.tensor.matmul(
                            out=ps[:],
                            lhsT=blkt[:, s, :],
                            rhs=ident[:],
                            start=(s == 0),
                            stop=(s == db - 1),
                        )
                    if which == "A":
                        # xin = (x + agg) as bf16, feature-major
                        xin = wpool.tile([64, BLK], mybir.dt.bfloat16,
                                         tag="xin")
                        nc.vector.tensor_add(
                            out=xin[:], in0=ps[:], in1=selfT_sb[:, bsl])
                        ps2 = p2pool.tile([64, BLK], mybir.dt.float32,
                                          space="PSUM")
                        nc.tensor.matmul(out=ps2[:], lhsT=ginW_sb[:],
                                         rhs=xin[:], start=True, stop=True)
                        hT = wpool.tile([64, BLK], mybir.dt.bfloat16,
                                        tag="hT")
                        nc.scalar.activation(
                            hT[:], ps2[:],
                            mybir.ActivationFunctionType.Relu,
                            bias=ginb_sb[:], scale=1.0)
                        ps3 = p2pool.tile([64, BLK], mybir.dt.float32,
                                          space="PSUM")
                        nc.tensor.matmul(out=ps3[:], lhsT=wcat_sb[:],
                                         rhs=hT[:], start=True, stop=True)
                        nc.vector.tensor_mul(
                            out=acc[:, bsl], in0=ps3[:],
                            in1=dinvT_sb[:, bsl])
                    else:
                        t1w = wpool.tile([64, BLK], mybir.dt.float32,
                                         tag="t1")
                        nc.vector.tensor_add(
                            out=t1w[:], in0=ps[:], in1=selfT_sb[:, bsl])
                        t2w = wpool.tile([64, BLK], mybir.dt.float32,
                                         tag="t2")
                        nc.vector.tensor_mul(
                            out=t2w[:], in0=t1w[:], in1=dinvT_sb[:, bsl])
                        # mu rows get relu, logvar rows pass through
                        nc.scalar.activation(
                            acc[0:COUT, bsl], t2w[0:COUT, :],
                            mybir.ActivationFunctionType.Relu,
                            bias=bias_sb[0:COUT, :], scale=1.0)
                        nc.vector.tensor_scalar_add(
                            acc[COUT:64, bsl], t2w[COUT:64, :],
                            bias_sb[COUT:64, :])
                nc.sync.dma_start(out=outT[:], in_=acc[:])
        nc.compile()
        from concourse.bass_interp import get_hw_module
        nc.m = get_hw_module(nc.m)
        return nc

    return build("A"), build("C")


def _prep(edge_index):
    """Shard/sort/pad the graph; returns per-core index structures."""
    src = np.asarray(edge_index[0], dtype=np.int64)
    dst = np.asarray(edge_index[1], dtype=np.int64)
    deg_in = np.bincount(dst, minlength=N)
    dinv = (1.0 / np.sqrt(deg_in + 1.0)).astype(np.float32)

    cores = []
    d_sched_per_core = np.zeros((NCORES, NBLK), dtype=np.int64)
    for c in range(NCORES):
        lo, hi = c * NPC, (c + 1) * NPC
        m = (dst >= lo) & (dst < hi)
        s_c = src[m]
        d_c = (dst[m] - lo).astype(np.int64)
        deg_c = np.bincount(d_c, minlength=NPC)
        order = np.argsort(deg_c, kind="stable")      # position -> local node
        pos = np.empty(NPC, dtype=np.int64)
        pos[order] = np.arange(NPC)                   # local node -> position
        posdeg = np.zeros(NPCP, dtype=np.int64)
        posdeg[:NPC] = deg_c[order]
        d_sched_per_core[c] = posdeg.reshape(NBLK, BLK).max(axis=1)
        cores.append((s_c, d_c, deg_c, order, pos, posdeg))

    d_sched = d_sched_per_core.max(axis=0)
    d_sched = np.maximum(d_sched, 1)
    t1 = int(d_sched.sum())
    tile_off = np.concatenate([[0], np.cumsum(d_sched)]).astype(np.int64)

    srcidx = np.full((NCORES, t1, BLK), -1, dtype=np.int64)
    pos_of_global = np.empty(N, dtype=np.int64)
    for c in range(NCORES):
        s_c, d_c, deg_c, order, pos, posdeg = cores[c]
        pos_of_global[c * NPC + order] = c * NPCP + np.arange(NPC)
        key = pos[d_c]
        eord = np.argsort(key, kind="stable")
        spos = key[eord]
        start_of_pos = np.zeros(NPCP, dtype=np.int64)
        np.cumsum(posdeg[:-1], out=start_of_pos[1:])
        r = np.arange(len(spos)) - start_of_pos[spos]
        t = tile_off[spos // BLK] + r
        srcidx[c, t, spos % BLK] = s_c[eord]
    return d_sched, t1, srcidx, pos_of_global, dinv, cores


TRACE = False
last_exec_ns = []


def _run(nc, in_maps):
    from concourse import bass_utils
    res = bass_utils.run_bass_kernel_spmd(nc, in_maps,
                                          core_ids=list(range(NCORES)),
                                          trace=TRACE)
    if TRACE:
        last_exec_ns.append(res.exec_time_ns)
    return res.results


def kernel(x, edge_index, gin_W, gin_b, mu_W, mu_b, lv_W, lv_b):
    x = np.asarray(x, dtype=np.float32)
    gin_W = np.asarray(gin_W, dtype=np.float32)
    gin_b = np.asarray(gin_b, dtype=np.float32)
    wcat = np.concatenate([np.asarray(mu_W, np.float32),
                           np.asarray(lv_W, np.float32)], axis=1)
    bias_cat = np.concatenate([np.asarray(mu_b, np.float32),
                               np.asarray(lv_b, np.float32)])

    d_sched, t1, srcidx, pos_of_global, dinv, cores = _prep(edge_index)

    key = ("prog", t1, tuple(int(v) for v in d_sched))
    if key not in _cache:
        _cache[key] = _build_programs(d_sched)
    nc_A, nc_C = _cache[key]

    # ---- launch A inputs ----
    x_pad = np.zeros((N + 1, 64), dtype=BF16)
    x_pad[:N] = x.astype(BF16)
    gather1 = np.where(srcidx >= 0, srcidx, N)

    in_maps_A = []
    for c in range(NCORES):
        _, _, _, order, _, _ = cores[c]
        xT = np.zeros((64, NPCP), dtype=np.float32)
        xT[:, :NPC] = x[c * NPC + order].T
        dT = np.ones((NPCP,), dtype=np.float32)
        dT[:NPC] = dinv[c * NPC + order]
        dinvT = np.broadcast_to(dT, (64, NPCP)).copy()
        in_maps_A.append({
            "slots": np.ascontiguousarray(
                x_pad[gather1[c]].transpose(1, 0, 2)),
            "selfT": xT,
            "dinvT": dinvT,
            "ginW": gin_W.astype(BF16),
            "ginb": gin_b.reshape(64, 1),
            "wcat": wcat.astype(BF16),
        })
    res_A = _run(nc_A, in_maps_A)

    # ---- assemble m table, build launch C inputs ----
    m_pos = np.zeros((NCORES * NPCP + 1, 64), dtype=BF16)
    for c in range(NCORES):
        m_pos[c * NPCP:(c + 1) * NPCP] = res_A[c]["outT"].T
    gather2 = np.where(srcidx >= 0, pos_of_global[srcidx],
                       NCORES * NPCP)

    in_maps_C = []
    for c in range(NCORES):
        in_maps_C.append({
            "slots": np.ascontiguousarray(
                m_pos[gather2[c]].transpose(1, 0, 2)),
            "selfT": m_pos[c * NPCP:(c + 1) * NPCP].T.astype(np.float32),
            "dinvT": in_maps_A[c]["dinvT"],
            "bias": bias_cat.reshape(64, 1),
        })
    res_C = _run(nc_C, in_maps_C)

    # ---- unshard ----
    out = np.empty((N, 64), dtype=np.float32)
    for c in range(NCORES):
        _, _, _, order, _, _ = cores[c]
        out[c * NPC + order] = res_C[c]["outT"][:, :NPC].T
    return out[:, :COUT], out[:, COUT:]


# revision 6
# speedup vs baseline: 1.2983x; 1.0394x over previous
"""GCN encoder (GIN conv -> 2x GCN conv) on 8 Trainium2 NeuronCores.

Strategy (dst-sharded, graph-parallel):
- Nodes are sharded by dst across 8 cores (12500 each). Each core owns the
  segment-sums and all dense math for its nodes; weights are replicated.
- Within a core, nodes are sorted by in-degree and grouped into 98 blocks of
  128; each block is padded to its max degree D_b, giving a dense
  [D_b, 128, 64] "slot" layout where tile s holds the s-th in-edge message of
  each of the 128 nodes. The segment-sum is then a chain of D_b TensorE
  matmuls accumulating into PSUM (lhsT = message tile, rhs = identity), which
  yields the aggregate directly in feature-major layout for the following
  linear layers.
- Per-edge message rows (x[src] for the GIN pass, the dinv-scaled
  concatenated GCN projections m[src] for the fused mu/logvar pass) are
  materialized into the slot layout on the host as part of input sharding;
  the device consumes them as dense streams at full DMA bandwidth.
- GCN normalization dinv[src]*dinv[dst] is factored: the table rows are
  pre-scaled by dinv[src] on device (launch A epilogue), and the dst factor
  is applied after the segment-sum (launch C epilogue), so no per-edge
  normalization gather is needed:
      out_i = dinv_i * (sum_{j->i} m_j + m_i) + b,   m_j = dinv_j * (h W)_j

Two SPMD launches:
  A: slots1 (x[src] rows) -> agg -> h = relu((x+agg) gin_W + gin_b)
     -> m = dinv * (h [mu_W|lv_W])          (per-core slice, feature-major)
  C: slots2 (m[src] rows) -> segment-sum -> epilogue -> [mu|logvar]
Host between launches: assemble the m table from the 8 slices and gather it
into the pass-2 slot layout (same index structure as pass 1).
"""

import numpy as np
import ml_dtypes

BF16 = ml_dtypes.bfloat16

N = 100000
E = 1600000
CIN = 64
HID = 64
COUT = 32
NCORES = 8
NPC = N // NCORES            # 12500 real nodes per core
BLK = 128
NBLK = 98                    # blocks per core
NPCP = NBLK * BLK            # 12544 padded positions per core

_cache = {}


def _build_programs(d_sched):
    import concourse.bass as bass
    import concourse.bacc as bacc
    import concourse.mybir as mybir
    import concourse.tile as tile
    from concourse.masks import make_identity

    t1 = int(np.sum(d_sched))
    tile_off = np.concatenate([[0], np.cumsum(d_sched)]).astype(int)

    def build(which):
        nc = bacc.Bacc("TRN2", target_bir_lowering=False, debug=False,
                       enable_asserts=False, num_devices=NCORES)
        slots = nc.dram_tensor("slots", [BLK, t1, 64], mybir.dt.bfloat16,
                               kind="ExternalInput").ap()
        selfT = nc.dram_tensor("selfT", [64, NPCP], mybir.dt.float32,
                               kind="ExternalInput").ap()
        dinvT = nc.dram_tensor("dinvT", [64, NPCP], mybir.dt.float32,
                               kind="ExternalInput").ap()
        if which == "A":
            ginW = nc.dram_tensor("ginW", [64, 64], mybir.dt.bfloat16,
                                  kind="ExternalInput").ap()
            ginb = nc.dram_tensor("ginb", [64, 1], mybir.dt.float32,
                                  kind="ExternalInput").ap()
            wcat = nc.dram_tensor("wcat", [64, 64], mybir.dt.bfloat16,
                                  kind="ExternalInput").ap()
            outT = nc.dram_tensor("outT", [64, NPCP], mybir.dt.bfloat16,
                                  kind="ExternalOutput").ap()
        else:
            bias = nc.dram_tensor("bias", [64, 1], mybir.dt.float32,
                                  kind="ExternalInput").ap()
            outT = nc.dram_tensor("outT", [64, NPCP], mybir.dt.float32,
                                  kind="ExternalOutput").ap()

        with tile.TileContext(nc) as tc:
            with (tc.tile_pool(name="const", bufs=1) as cpool,
                  tc.tile_pool(name="blkin", bufs=3) as bpool,
                  tc.tile_pool(name="work", bufs=4) as wpool,
                  tc.tile_pool(name="ps", bufs=3, space="PSUM") as ppool,
                  tc.tile_pool(name="ps2", bufs=2, space="PSUM") as p2pool):
                ident = cpool.tile([BLK, BLK], mybir.dt.bfloat16)
                make_identity(nc, ident[:])
                CHB = 7                       # blocks per const chunk
                CHW = CHB * BLK
                selfT_sb = []
                dinvT_sb = []
                for k in range(NBLK // CHB):
                    st = cpool.tile([64, CHW], mybir.dt.float32,
                                    tag=f"selfT{k}")
                    nc.sync.dma_start(out=st[:],
                                      in_=selfT[:, k * CHW:(k + 1) * CHW])
                    selfT_sb.append(st)
                    dt_ = cpool.tile([64, CHW], mybir.dt.float32,
                                     tag=f"dinvT{k}")
                    nc.sync.dma_start(out=dt_[:],
                                      in_=dinvT[:, k * CHW:(k + 1) * CHW])
                    dinvT_sb.append(dt_)
                if which == "A":
                    ginW_sb = cpool.tile([64, 64], mybir.dt.bfloat16)
                    nc.sync.dma_start(out=ginW_sb[:], in_=ginW[:])
                    ginb_sb = cpool.tile([64, 1], mybir.dt.float32)
                    nc.sync.dma_start(out=ginb_sb[:], in_=ginb[:])
                    wcat_sb = cpool.tile([64, 64], mybir.dt.bfloat16)
                    nc.sync.dma_start(out=wcat_sb[:], in_=wcat[:])
                    acc = cpool.tile([64, NPCP], mybir.dt.bfloat16)
                else:
                    bias_sb = cpool.tile([64, 1], mybir.dt.float32)
                    nc.sync.dma_start(out=bias_sb[:], in_=bias[:])
                    acc = cpool.tile([64, NPCP], mybir.dt.float32)

                dmax = int(np.max(d_sched))
                for b in range(NBLK):
                    db = int(d_sched[b])
                    bsl = slice(b * BLK, (b + 1) * BLK)
                    csl = slice((b % CHB) * BLK, (b % CHB + 1) * BLK)
                    selfT_b = selfT_sb[b // CHB]
                    dinvT_b = dinvT_sb[b // CHB]
                    # one DMA brings the whole block's slot tiles:
                    # DRAM [db, 128, 64] -> SBUF [128, db, 64]
                    blkt = bpool.tile([BLK, dmax, 64], mybir.dt.bfloat16,
                                      tag="blk")
                    nc.sync.dma_start(
                        out=blkt[:, :db, :],
                        in_=slots[:, tile_off[b]:tile_off[b] + db, :],
                    )
                    ps = ppool.tile([64, BLK], mybir.dt.float32, space="PSUM")
                    for s in range(db):
                        nc.tensor.matmul(
                            out=ps[:],
                            lhsT=blkt[:, s, :],
                            rhs=ident[:],
                            start=(s == 0),
                            stop=(s == db - 1),
                        )
                    if which == "A":
                        # xin = (x + agg) as bf16, feature-major
                        xin = wpool.tile([64, BLK], mybir.dt.bfloat16,
                                         tag="xin")
                        nc.vector.tensor_add(
                            out=xin[:], in0=ps[:], in1=selfT_b[:, csl])
                        ps2 = p2pool.tile([64, BLK], mybir.dt.float32,
                                          space="PSUM")
                        nc.tensor.matmul(out=ps2[:], lhsT=ginW_sb[:],
                                         rhs=xin[:], start=True, stop=True)
                        hT = wpool.tile([64, BLK], mybir.dt.bfloat16,
                                        tag="hT")
                        nc.scalar.activation(
                            hT[:], ps2[:],
                            mybir.ActivationFunctionType.Relu,
                            bias=ginb_sb[:], scale=1.0)
                        ps3 = p2pool.tile([64, BLK], mybir.dt.float32,
                                          space="PSUM")
                        nc.tensor.matmul(out=ps3[:], lhsT=wcat_sb[:],
                                         rhs=hT[:], start=True, stop=True)
                        nc.vector.tensor_mul(
                            out=acc[:, bsl], in0=ps3[:],
                            in1=dinvT_b[:, csl])
                    else:
                        t1w = wpool.tile([64, BLK], mybir.dt.float32,
                                         tag="t1")
                        nc.vector.tensor_add(
                            out=t1w[:], in0=ps[:], in1=selfT_b[:, csl])
                        t2w = wpool.tile([64, BLK], mybir.dt.float32,
                                         tag="t2")
                        nc.vector.tensor_mul(
                            out=t2w[:], in0=t1w[:], in1=dinvT_b[:, csl])
                        # mu rows get relu, logvar rows pass through
                        nc.vector.tensor_scalar(
                            out=acc[0:COUT, bsl], in0=t2w[0:COUT, :],
                            scalar1=bias_sb[0:COUT, :], scalar2=0.0,
                            op0=mybir.AluOpType.add,
                            op1=mybir.AluOpType.max)
                        nc.vector.tensor_scalar_add(
                            acc[COUT:64, bsl], t2w[COUT:64, :],
                            bias_sb[COUT:64, :])
                for k in range(NBLK // CHB):
                    nc.sync.dma_start(
                        out=outT[:, k * CHW:(k + 1) * CHW],
                        in_=acc[:, k * CHW:(k + 1) * CHW])
        nc.compile()
        from concourse.bass_interp import get_hw_module
        nc.m = get_hw_module(nc.m)
        return nc

    return build("A"), build("C")


def _prep(edge_index):
    """Shard/sort/pad the graph; returns per-core index structures."""
    src = np.asarray(edge_index[0], dtype=np.int64)
    dst = np.asarray(edge_index[1], dtype=np.int64)
    deg_in = np.bincount(dst, minlength=N)
    dinv = (1.0 / np.sqrt(deg_in + 1.0)).astype(np.float32)

    cores = []
    d_sched_per_core = np.zeros((NCORES, NBLK), dtype=np.int64)
    for c in range(NCORES):
        lo, hi = c * NPC, (c + 1) * NPC
        m = (dst >= lo) & (dst < hi)
        s_c = src[m]
        d_c = (dst[m] - lo).astype(np.int64)
        deg_c = np.bincount(d_c, minlength=NPC)
        order = np.argsort(deg_c, kind="stable")      # position -> local node
        pos = np.empty(NPC, dtype=np.int64)
        pos[order] = np.arange(NPC)                   # local node -> position
        posdeg = np.zeros(NPCP, dtype=np.int64)
        posdeg[:NPC] = deg_c[order]
        d_sched_per_core[c] = posdeg.reshape(NBLK, BLK).max(axis=1)
        cores.append((s_c, d_c, deg_c, order, pos, posdeg))

    d_sched = d_sched_per_core.max(axis=0)
    d_sched = np.maximum(d_sched, 1)
    t1 = int(d_sched.sum())
    tile_off = np.concatenate([[0], np.cumsum(d_sched)]).astype(np.int64)

    srcidx = np.full((NCORES, t1, BLK), -1, dtype=np.int64)
    pos_of_global = np.empty(N, dtype=np.int64)
    for c in range(NCORES):
        s_c, d_c, deg_c, order, pos, posdeg = cores[c]
        pos_of_global[c * NPC + order] = c * NPCP + np.arange(NPC)
        key = pos[d_c]
        eord = np.argsort(key, kind="stable")
        spos = key[eord]
        start_of_pos = np.zeros(NPCP, dtype=np.int64)
        np.cumsum(posdeg[:-1], out=start_of_pos[1:])
        r = np.arange(len(spos)) - start_of_pos[spos]
        t = tile_off[spos // BLK] + r
        srcidx[c, t, spos % BLK] = s_c[eord]
    return d_sched, t1, srcidx, pos_of_global, dinv, cores


TRACE = False
last_exec_ns = []


def _run(nc, in_maps):
    from concourse import bass_utils
    res = bass_utils.run_bass_kernel_spmd(nc, in_maps,
                                          core_ids=list(range(NCORES)),
                                          trace=TRACE)
    if TRACE:
        last_exec_ns.append(res.exec_time_ns)
    return res.results


def kernel(x, edge_index, gin_W, gin_b, mu_W, mu_b, lv_W, lv_b):
    x = np.asarray(x, dtype=np.float32)
    gin_W = np.asarray(gin_W, dtype=np.float32)
    gin_b = np.asarray(gin_b, dtype=np.float32)
    wcat = np.concatenate([np.asarray(mu_W, np.float32),
                           np.asarray(lv_W, np.float32)], axis=1)
    bias_cat = np.concatenate([np.asarray(mu_b, np.float32),
                               np.asarray(lv_b, np.float32)])

    d_sched, t1, srcidx, pos_of_global, dinv, cores = _prep(edge_index)

    key = ("prog", t1, tuple(int(v) for v in d_sched))
    if key not in _cache:
        _cache[key] = _build_programs(d_sched)
    nc_A, nc_C = _cache[key]

    # ---- launch A inputs ----
    x_pad = np.zeros((N + 1, 64), dtype=BF16)
    x_pad[:N] = x.astype(BF16)
    gather1 = np.where(srcidx >= 0, srcidx, N)

    in_maps_A = []
    for c in range(NCORES):
        _, _, _, order, _, _ = cores[c]
        xT = np.zeros((64, NPCP), dtype=np.float32)
        xT[:, :NPC] = x[c * NPC + order].T
        dT = np.ones((NPCP,), dtype=np.float32)
        dT[:NPC] = dinv[c * NPC + order]
        dinvT = np.broadcast_to(dT, (64, NPCP)).copy()
        in_maps_A.append({
            "slots": np.ascontiguousarray(
                x_pad[gather1[c]].transpose(1, 0, 2)),
            "selfT": xT,
            "dinvT": dinvT,
            "ginW": gin_W.astype(BF16),
            "ginb": gin_b.reshape(64, 1),
            "wcat": wcat.astype(BF16),
        })
    res_A = _run(nc_A, in_maps_A)

    # ---- assemble m table, build launch C inputs ----
    m_pos = np.zeros((NCORES * NPCP + 1, 64), dtype=BF16)
    for c in range(NCORES):
        m_pos[c * NPCP:(c + 1) * NPCP] = res_A[c]["outT"].T
    gather2 = np.where(srcidx >= 0, pos_of_global[srcidx],
                       NCORES * NPCP)

    in_maps_C = []
    for c in range(NCORES):
        in_maps_C.append({
            "slots": np.ascontiguousarray(
                m_pos[gather2[c]].transpose(1, 0, 2)),
            "selfT": m_pos[c * NPCP:(c + 1) * NPCP].T.astype(np.float32),
            "dinvT": in_maps_A[c]["dinvT"],
            "bias": bias_cat.reshape(64, 1),
        })
    res_C = _run(nc_C, in_maps_C)

    # ---- unshard ----
    out = np.empty((N, 64), dtype=np.float32)
    for c in range(NCORES):
        _, _, _, order, _, _ = cores[c]
        out[c * NPC + order] = res_C[c]["outT"][:, :NPC].T
    return out[:, :COUT], out[:, COUT:]
